# revision 1
# baseline (speedup 1.0000x reference)
"""BiDAF attention (nn_BertBidafAttention) on 8 TRN2 NeuronCores.

Math (per batch, reference):
    cp = c @ W.T + b            [CL, H]
    s  = cp @ q.T               [CL, QL]
    s1 = softmax_q(s + qmask_bias)      (row softmax)
    s2 = softmax_c(s + cmask_bias)      (col softmax)
    a  = s1 @ q                 [CL, H]
    bv = (s1 @ s2.T) @ c        [CL, H]
    x  = [c, a, c*a, c*bv]      [CL, 4H]

Restructured:
    qWT[h, q] = sum_d W[d, h] qT[d, q]   (75 MF vs 604 MF for cp), fp16
    sT[q, c]  = sum_h qWT[h, q] cT[h, c] + cmask_bias[c]   (f32r, 512-free)
    softmaxes from sT; value matmuls (a, bv, qc) in fp16/f32r.
The c-mask bias is constant along q so it cancels in s1's softmax; qb =
q @ b is constant along c so it cancels in s2's.  NEGB = -1000 makes
exp(masked - max) == 0 exactly in fp32.

The output's first quarter is just c, so the device only computes/stores
[a, c*a, c*bv] ([CL, 3H]); the host splices the input c back in.  All
inputs are issued up front (scalar: q/W/b, sync: c, pool: masks); batch
1's cT transposes are interleaved into batch 0's value loop so the PE
never idles; stores stream per 128-row tile (batch 0 on the pool queue
so its SWDGE drain happens early, batch 1 on sync).

Sharding: data-parallel over batch, 2 batches per core, no collectives.
"""

import numpy as np
from contextlib import ExitStack

import concourse.bass as bass
from concourse import bacc
import concourse.mybir as mybir
import concourse.tile as tile
from concourse.masks import make_identity
from concourse.bass_utils import run_bass_kernel_spmd

B, CL, QL, H = 16, 512, 64, 768
NCORES = 8
BPC = B // NCORES  # batches per core
HK = H // 128      # 6 chunks over feature dims
CT = CL // 128     # 4 c-tiles
TH = 3 * H         # device output row: [a, c*a, c*bv]
NEGB = -1000.0     # additive mask bias; exp(NEGB - max) == 0.0 in fp32

f32 = mybir.dt.float32
f32r = mybir.dt.float32r
f16 = mybir.dt.float16
i32 = mybir.dt.int32
EXP = mybir.ActivationFunctionType.Exp
COPY = mybir.ActivationFunctionType.Copy
AXF = mybir.AxisListType.X


def _build_nc(precision: int = 1, repeat: int = 1, hwloop: int = 0) -> bass.Bass:
    nc = bacc.Bacc()
    cD = nc.declare_dram_parameter("c", [BPC, CL, H], f32, isOutput=False)
    qD = nc.declare_dram_parameter("q", [BPC, QL, H], f32, isOutput=False)
    cmD = nc.declare_dram_parameter("c_mask", [BPC, CL], i32, isOutput=False)
    qmD = nc.declare_dram_parameter("q_mask", [BPC, QL], i32, isOutput=False)
    WD = nc.declare_dram_parameter("W", [H, H], f32, isOutput=False)
    bD = nc.declare_dram_parameter("b", [H], f32, isOutput=False)
    outD = nc.declare_dram_parameter("out", [BPC, CL, TH], f32, isOutput=True)

    def asf32(ap):
        return ap.bitcast(f32) if ap.dtype != f32 else ap

    with tile.TileContext(nc) as tc, ExitStack() as ctx:
        const = ctx.enter_context(tc.tile_pool(name="const", bufs=1))
        wpool = ctx.enter_context(tc.tile_pool(name="wpool", bufs=1))
        small = ctx.enter_context(tc.tile_pool(name="small", bufs=2))
        outp = ctx.enter_context(tc.tile_pool(name="outp", bufs=2))
        pp = ctx.enter_context(tc.tile_pool(name="pp", bufs=1, space="PSUM"))

        # --- constants ---
        ident = const.tile([128, 128], f32)
        make_identity(nc, ident)
        ident16 = const.tile([128, 128], f16)
        nc.vector.tensor_copy(out=ident16, in_=ident)

        ones = const.tile([1, 128], f32)
        nc.vector.memset(ones, 1.0)
        # f32r tiles must be produced as f32r (copy/DMA), not bitcast views
        onesR = const.tile([1, 128], f32r)
        nc.vector.tensor_copy(out=onesR, in_=ones)

        # --- input DMAs: all on the sync queue (it has no early compute),
        # in consumption order: q packed, W, b, c b0, c b1, q per-batch ---
        qp = wpool.tile([128, H], f32)
        nc.sync.dma_start(out=qp, in_=qD[:].rearrange("b q h -> (b q) h"))
        b_st = wpool.tile([128, HK], f32)
        nc.sync.dma_start(out=b_st, in_=bD[:].rearrange("(k p) -> p k", p=128))
        w_sb = wpool.tile([128, HK, H], f32)
        for k in range(HK):
            nc.sync.dma_start(out=w_sb[:, k, :],
                              in_=WD[k * 128:(k + 1) * 128, :])
        c_nat = [wpool.tile([128, CT, H], f32r, name=f"cn{i}")
                 for i in range(BPC)]
        for ci in (2, 3):
            nc.sync.dma_start(
                out=c_nat[0][:, ci, :],
                in_=cD[0, ci * 128:(ci + 1) * 128, :].bitcast(f32r))
        for ci in range(CT):
            nc.sync.dma_start(
                out=c_nat[1][:, ci, :],
                in_=cD[1, ci * 128:(ci + 1) * 128, :].bitcast(f32r))
        # gpsimd queue: masks, c b0 front half, q per-batch; b0 stores later
        qmfc = small.tile([QL, BPC], f32, tag="qmfc", bufs=1)
        nc.gpsimd.dma_start(out=qmfc, in_=qmD[:].rearrange("b l -> l b"))
        cmf = small.tile([1, BPC, CL], f32, tag="cmf", bufs=1)
        nc.gpsimd.dma_start(out=cmf[:1].rearrange("o b l -> o (b l)"),
                            in_=cmD[:].rearrange("b (o l) -> o (b l)", o=1))
        for ci in (0, 1):
            nc.gpsimd.dma_start(
                out=c_nat[0][:, ci, :],
                in_=cD[0, ci * 128:(ci + 1) * 128, :].bitcast(f32r))
        q_f = []
        for bi in range(BPC):
            qf = small.tile([QL, H], f32, tag="qf")
            nc.gpsimd.dma_start(out=qf, in_=qD[bi])
            q_f.append(qf)

        # --- hot path: q16p/b16 casts -> qT2 transposes (PE starts ASAP) ---
        q16p = wpool.tile([128, H], f16)
        nc.vector.tensor_copy(out=q16p, in_=qp)
        b16 = wpool.tile([128, HK], f16)
        nc.vector.tensor_copy(out=b16, in_=b_st)
        qT2 = wpool.tile([128, HK, 128], f16)
        ptq = pp.tile([128, HK, 128], f16, tag="ctp", bufs=2, name="ptq")
        for k in range(HK):
            nc.tensor.transpose(ptq[:, k, :], q16p[:, k * 128:(k + 1) * 128],
                                ident16)
        nc.vector.tensor_copy(out=qT2, in_=ptq)

        # --- w16 casts (DVE, paced by W arrival) ---
        w16 = wpool.tile([128, HK, H], f16)
        for k in range(HK):
            nc.scalar.copy(out=w16[:, k, :], in_=w_sb[:, k, :])
        q16v = []
        for bi in range(BPC):
            q1 = wpool.tile([QL, H], f16, name=f"q16v{bi}")
            nc.vector.tensor_copy(out=q1, in_=q_f[bi])
            q16v.append(q1)

        # --- mask biases: (mask - 1) * |NEGB| ---
        qbias_c = small.tile([QL, BPC], f32, tag="qbias_c", bufs=1)
        nc.scalar.activation(qbias_c, qmfc, COPY, bias=NEGB, scale=-NEGB)
        cbias = small.tile([1, BPC, CL], f32, tag="cbias", bufs=1)
        nc.scalar.activation(cbias, cmf, COPY, bias=NEGB, scale=-NEGB)
        cbiasR = small.tile([1, BPC, CL], f32r, tag="cbiasR", bufs=1)
        nc.vector.tensor_copy(out=cbiasR, in_=cbias)

        # --- qb[q] = q @ b (+ q-mask bias) per batch ---
        qrc = []
        for bi in range(BPC):
            # shares the "pss" bank slot (same 1024B/partition)
            pqb = pp.tile([QL, 512], f32, tag="pst", bufs=1,
                          name=f"pqb{bi}")
            for k in range(HK):
                nc.tensor.matmul(pqb[:, 0:1],
                                 qT2[:, k, bi * QL:(bi + 1) * QL],
                                 b16[:, k:k + 1],
                                 start=(k == 0), stop=(k == HK - 1))
            r = small.tile([QL, 1], f32, tag=f"qrc{bi}", bufs=1)
            nc.vector.tensor_add(r, pqb[:, 0:1], qbias_c[:, bi:bi + 1])
            qrc.append(r)

        # --- cT via PE transposes; batch-0 copies ride the idle Scalar ---
        cT = [wpool.tile([128, HK, CL], f32r, name=f"cT{i}")
              for i in range(BPC)]

        def emit_cT_ci(bi, ci, copy_eng):
            for half in range(2):
                ctp = pp.tile([128, 3, 128], f32, tag="ctp", bufs=2)
                for j, k in enumerate(range(half * 3, half * 3 + 3)):
                    nc.tensor.transpose(
                        ctp[:, j, :],
                        asf32(c_nat[bi][:, ci, k * 128:(k + 1) * 128]),
                        ident)
                dst = cT[bi][:, half * 3:half * 3 + 3,
                             ci * 128:(ci + 1) * 128]
                if copy_eng is nc.scalar:
                    nc.scalar.copy(out=dst, in_=ctp)
                else:
                    copy_eng.tensor_copy(out=dst, in_=ctp)

        for ci in range(CT):
            emit_cT_ci(0, ci, nc.scalar if ci < 2 else nc.vector)

        # --- qWT (fp16): hm-outer, k-inner ---
        qwt = wpool.tile([128, HK, 128], f32r)
        for hm in range(HK):
            pw = pp.tile([128, 128], f32, tag="pw", bufs=1)
            for k in range(HK):
                nc.tensor.matmul(pw, w16[:, k, hm * 128:(hm + 1) * 128],
                                 qT2[:, k, :],
                                 start=(k == 0), stop=(k == HK - 1))
            nc.scalar.copy(out=qwt[:, hm, :], in_=pw)

        # --- sT accumulation + tails ---
        pst = [None, None]

        def emit_sT(bi):
            pst[bi] = pp.tile([QL, CL], f32, tag="pst", bufs=1,
                              name=f"pst{bi}")
            for hm in range(HK):
                nc.tensor.matmul(pst[bi], qwt[:, hm, bi * QL:(bi + 1) * QL],
                                 cT[bi][:, hm, :],
                                 start=(hm == 0), stop=False)
            nc.tensor.matmul(pst[bi], onesR[:1, :QL], cbiasR[:1, bi],
                             start=False, stop=True)

        emit_sT(0)

        def emit_tail(bi, per_ci_hook=None):
            pstb = pst[bi]
            # s2: softmax over c (free axis of sT); qb cancels here
            nmax2 = small.tile([QL, 1], f32, tag="nmax2")
            nc.vector.reduce_max(nmax2, pstb, axis=AXF, negate=True)
            s2e = small.tile([QL, CL], f32, tag="s2e")
            sum2 = small.tile([QL, 1], f32, tag="sum2")
            nc.scalar.activation(s2e, pstb, EXP, bias=nmax2, scale=1.0,
                                 accum_out=sum2)
            r2 = small.tile([QL, 1], f32, tag="r2")
            nc.vector.reciprocal(r2, sum2)
            # s2 normalization is deferred into the qc16 scale below
            ps2 = pp.tile([128, CT * QL], f32, tag="pss", bufs=1)
            for ci in range(CT):
                nc.tensor.transpose(ps2[:, ci * QL:(ci + 1) * QL],
                                    s2e[:, ci * 128:(ci + 1) * 128],
                                    ident[:QL, :QL])
            s2sb = small.tile([128, CT, QL], f32r, tag="s2sb")
            nc.vector.tensor_copy(
                out=s2sb, in_=ps2.rearrange("p (c q) -> p c q", c=CT))

            # s1: softmax over q; c-mask bias cancels here
            sTb = small.tile([QL, CL], f32, tag="sTb")
            nc.vector.tensor_scalar_add(sTb, pstb, qrc[bi])
            ps_s = pp.tile([128, CT * QL], f32, tag="pss", bufs=1)
            for ci in range(CT):
                nc.tensor.transpose(ps_s[:, ci * QL:(ci + 1) * QL],
                                    sTb[:, ci * 128:(ci + 1) * 128],
                                    ident[:QL, :QL])
            s1n = small.tile([128, CT, QL], f16, tag="s1n")
            for ci in range(CT):
                sl = ps_s[:, ci * QL:(ci + 1) * QL]
                nmax1 = small.tile([128, 1], f32, tag="nmax1")
                nc.vector.reduce_max(nmax1, sl, axis=AXF, negate=True)
                e1 = small.tile([128, QL], f32, tag="e1")
                sum1 = small.tile([128, 1], f32, tag="sum1")
                nc.scalar.activation(e1, sl, EXP, bias=nmax1, scale=1.0,
                                     accum_out=sum1)
                r1 = small.tile([128, 1], f32, tag="r1")
                nc.vector.reciprocal(r1, sum1)
                nc.vector.tensor_scalar_mul(s1n[:, ci, :], e1, r1)
            ps1t = pp.tile([QL, CT * 128], f16, tag="pss", bufs=1)
            for ci in range(CT):
                nc.tensor.transpose(ps1t[:, ci * 128:(ci + 1) * 128],
                                    s1n[:, ci, :], ident16)
            s1T = small.tile([QL, CT * 128], f16, tag="s1T")
            nc.vector.tensor_copy(out=s1T, in_=ps1t)

            # qc[q, h] = s2.T @ c  (f32r), then fp16 for the bv matmuls
            qc16 = small.tile([QL, H], f16, tag="qc16")
            for hf, (lo, sz) in enumerate(((0, 512), (512, 256))):
                # shares the "pst" bank slot (same 2048B/partition)
                pqc = pp.tile([QL, 512], f32, tag="psv", bufs=3,
                              name=f"pqc{bi}_{hf}")
                for ci in range(CT):
                    nc.tensor.matmul(pqc[:, 0:sz], s2sb[:, ci, :],
                                     c_nat[bi][:, ci, lo:lo + sz],
                                     start=(ci == 0), stop=(ci == CT - 1))
                nc.vector.tensor_scalar_mul(qc16[:, lo:lo + sz],
                                             pqc[:, 0:sz], r2)

            # a = s1 @ q ; bv = s1 @ qc ; out tile = [a, c*a, c*bv]
            for ci in range(CT):
                osb = outp.tile([128, TH], f32, tag="osb", bufs=3)
                lhs = s1T[:, ci * 128:(ci + 1) * 128]
                csl = c_nat[bi][:, ci, :]
                for lo, sz in ((0, 512), (512, 256)):
                    pa = pp.tile([128, 512], f32, tag="psv", bufs=3)
                    nc.tensor.matmul(pa[:, 0:sz], lhs,
                                     q16v[bi][:, lo:lo + sz],
                                     start=True, stop=True)
                    pb = pp.tile([128, 512], f32, tag="psv", bufs=3)
                    nc.tensor.matmul(pb[:, 0:sz], lhs, qc16[:, lo:lo + sz],
                                     start=True, stop=True)
                    nc.scalar.copy(out=osb[:, lo:lo + sz], in_=pa[:, 0:sz])
                    nc.vector.tensor_mul(osb[:, H + lo:H + lo + sz],
                                         asf32(csl[:, lo:lo + sz]),
                                         pa[:, 0:sz])
                    nc.vector.tensor_mul(osb[:, 2 * H + lo:2 * H + lo + sz],
                                         asf32(csl[:, lo:lo + sz]),
                                         pb[:, 0:sz])
                eng = nc.gpsimd if bi == 0 else nc.sync
                eng.dma_start(out=outD[bi, ci * 128:(ci + 1) * 128, :],
                              in_=osb)
                if per_ci_hook is not None:
                    per_ci_hook(ci)

        def b1_prep(ci):
            emit_cT_ci(1, ci, nc.vector)
            if ci == CT - 1:
                emit_sT(1)

        emit_tail(0, per_ci_hook=b1_prep)
        emit_tail(1)

    nc.finalize()
    return nc


_NC_CACHE: dict = {}


def _get_nc(precision: int = 1) -> bass.Bass:
    if precision not in _NC_CACHE:
        _NC_CACHE[precision] = _build_nc(precision)
    return _NC_CACHE[precision]


def kernel(c, q, c_mask, q_mask, W, b, _trace=False, _precision=1):
    nc = _get_nc(_precision)
    in_maps = []
    for i in range(NCORES):
        sl = slice(i * BPC, (i + 1) * BPC)
        in_maps.append({
            "c": np.ascontiguousarray(np.asarray(c)[sl], dtype=np.float32),
            "q": np.ascontiguousarray(np.asarray(q)[sl], dtype=np.float32),
            "c_mask": np.ascontiguousarray(np.asarray(c_mask)[sl], dtype=np.int32),
            "q_mask": np.ascontiguousarray(np.asarray(q_mask)[sl], dtype=np.int32),
            "W": np.ascontiguousarray(np.asarray(W), dtype=np.float32),
            "b": np.ascontiguousarray(np.asarray(b), dtype=np.float32),
        })
    res = run_bass_kernel_spmd(nc, in_maps, core_ids=list(range(NCORES)),
                               trace=_trace)
    dev = np.concatenate([res.results[i]["out"] for i in range(NCORES)], axis=0)
    out = np.empty((B, CL, 4 * H), dtype=np.float32)
    out[:, :, :H] = np.asarray(c, dtype=np.float32)
    out[:, :, H:] = dev
    if _trace:
        return out, res
    return out

